# revision 26
# baseline (speedup 1.0000x reference)
"""Trainium2 Bass kernel for the two-layer LIF spiking network.

Model (snntorch Leaky, reset_mechanism='subtract', beta=0.9):
    mem1 = beta*mem1 + (x_t @ w1.T + b1) - spk1_prev*thr1 ; spk1 = (mem1-thr1 > 0)
    mem2 = beta*mem2 + (spk1 @ w2.T + b2) - spk2_prev*thr2 ; spk2 = (mem2-thr2 > 0)

Strategy (8 NeuronCores, data-parallel over batch, B_loc = 16 per core):
  * Layer-1 currents for ALL timesteps are one big matmul (x known ahead);
    layer-2 currents for a time-block are a matmul of that block's spikes.
    Only the cheap elementwise membrane update is sequential.
  * f32 matmul on PE runs at 1/4 rate; instead split operands to fp16 hi+lo
    on the host and run 3 fp16 passes (wh*xh + wl*xh + wh*xl): ~2e-6 error
    at 3/4 of the f32 cost. Spikes are exactly representable in fp16, so
    matmul-2 needs only 2 passes (w2h, w2l).
  * The membrane scan is ONE custom fused DVE op per timestep:
        m' = (m*beta + cur) - (m > thr)*thr
    which is bit-exact the reference's f32 rounding order. Spikes are
    extracted per-block in bulk (is_gt) straight into the fp16 tile that
    feeds matmul-2 and the output DMA.
  * Layout: [128(par), slot, t*B_loc] slabs everywhere; matmul with K=i (or
    h) on partitions produces [h, (t,b)] directly; scans walk 16-column
    (one timestep) slices. No transposes anywhere; host reassembles.
  * Emission is software-pipelined (mm1 of block j+1 before mm2 of block j)
    so the PE stream never stalls on the scans.
"""

import os
import sys
import types

import numpy as np

_TRN_REPO = "/opt/trn_rl_repo"
if _TRN_REPO not in sys.path:
    sys.path.insert(0, _TRN_REPO)


def _install_axon_ntff_hook():
    """The image's antenv package lacks axon_hooks; provide it so
    run_bass_kernel_spmd(trace=True) can capture NTFF profiles."""
    if "antenv.axon_hooks" in sys.modules:
        return
    try:
        import antenv
    except ImportError:
        return
    mod = types.ModuleType("antenv.axon_hooks")
    mod._HOOK = None

    def set_axon_ntff_profile_hook(hook):
        mod._HOOK = hook

    def get_axon_ntff_profile_hook():
        if mod._HOOK is None:
            try:
                from trn_agent_boot.trn_boot import _ntff_profile_via_ctypes

                mod._HOOK = _ntff_profile_via_ctypes("/opt/axon/libaxon_pjrt.so")
            except Exception:
                mod._HOOK = None
        return mod._HOOK

    mod.set_axon_ntff_profile_hook = set_axon_ntff_profile_hook
    mod.get_axon_ntff_profile_hook = get_axon_ntff_profile_hook
    sys.modules["antenv.axon_hooks"] = mod
    antenv.axon_hooks = mod


_install_axon_ntff_hook()

from concourse import bacc, mybir, tile  # noqa: E402
from concourse.alu_op_type import AluOpType  # noqa: E402
from concourse.bass_utils import run_bass_kernel_spmd  # noqa: E402

AF = mybir.ActivationFunctionType
F16 = mybir.dt.float16
F32 = mybir.dt.float32

# Problem constants (hardcoded per spec).
T, B, I, H, O = 256, 128, 1024, 1024, 256
BETA = 0.9
NCORES = 8
B_LOC = B // NCORES            # 16
TB = T * B_LOC                 # 4096 columns (t-major, b-minor)
KT1, MT1 = I // 128, H // 128  # 8, 8
KT2, MT2 = H // 128, O // 128  # 8, 2
BLK_STEPS = 32                 # timesteps per standard pipeline block
BLK = BLK_STEPS * B_LOC        # 512 columns per block
NBLK = T // BLK_STEPS          # 8
# The first two blocks are half-size so the first matmul group is gated by
# ~3 MB of DMA instead of ~6 MB; the rest are full 32-step blocks.
_SIZES = [8, 8, 16] + [32] * 6 + [16, 8, 8]
BLOCKS = [(sum(_SIZES[:i]) * B_LOC, _SIZES[i]) for i in range(len(_SIZES))]
assert sum(_SIZES) == T

LAST_EXEC_NS = None


def _register_lif_op():
    """Custom DVE op: out = (in0*s0 + in1) - (in0 > s1)*s1.

    With s0=beta, s1=thr and in1 the (bias-included) input current this is
    one LIF membrane update, with the same fp32 rounding order as the
    reference (verified bit-exact on HW)."""
    import concourse.dve_ops as dve_ops_mod
    from concourse.dve_spec import C0, C1, Spec, Src0, Src1, lower
    from concourse.dve_uop import DveOpSpec

    name = "LIF_STEP_ANT"
    for op in dve_ops_mod.OPS:
        if op.name == name:
            return op
    body = (Src0 * C0 + Src1) - (Src0 > C1) * C1

    def ref(in0, in1, s0, s1, imm2):
        return (in0.astype(np.float32) * s0 + in1) - (
            in0 > s1).astype(np.float32) * s1

    spec = Spec(body=body, reference=ref)
    shas = {}
    for ver in ("v3", "v4"):
        tmp = DveOpSpec(name=name, opcode=None, uops=lower(spec, ver=ver),
                        rd1_en=True)
        shas[ver] = tmp.sha(ver)
    op = dve_ops_mod.DveOp(name, spec, subdim=False, uops_sha=shas)
    row = max(dve_ops_mod._SUB_OPCODE_FOR_NAME.values()) + 1
    assert row < 0x20
    dve_ops_mod.OPS.append(op)
    dve_ops_mod._SUB_OPCODE_FOR_NAME[name] = row
    dve_ops_mod.CUSTOM_DVE_SPECS[name] = spec
    return op


LIF = _register_lif_op()


def _build(thr1: float, thr2: float):
    nc = bacc.Bacc("TRN2", target_bir_lowering=False, debug=False,
                   num_devices=NCORES)

    xh_d = nc.dram_tensor("xh", [I, TB], F16, kind="ExternalInput")
    xl_d = nc.dram_tensor("xl", [I, TB], F16, kind="ExternalInput")
    w1h_d = nc.dram_tensor("w1h", [I, H], F16, kind="ExternalInput")
    w1l_d = nc.dram_tensor("w1l", [I, H], F16, kind="ExternalInput")
    w2h_d = nc.dram_tensor("w2h", [H, O], F16, kind="ExternalInput")
    w2l_d = nc.dram_tensor("w2l", [H, O], F16, kind="ExternalInput")
    b1_d = nc.dram_tensor("b1", [MT1, 128], F32, kind="ExternalInput")
    b2_d = nc.dram_tensor("b2", [MT2, 128], F32, kind="ExternalInput")
    s1_d = nc.dram_tensor("s1", [MT1, 128, TB], F16, kind="ExternalOutput")
    s2_d = nc.dram_tensor("s2", [MT2, 128, TB], F16, kind="ExternalOutput")

    with tile.TileContext(nc) as tc:
        import contextlib

        ctx = contextlib.ExitStack()
        with ctx:
            wpool = ctx.enter_context(tc.tile_pool(name="weights", bufs=1))
            xpool = ctx.enter_context(tc.tile_pool(name="x", bufs=3))
            c1pool = ctx.enter_context(tc.tile_pool(name="cur1", bufs=2))
            c2pool = ctx.enter_context(tc.tile_pool(name="cur2", bufs=2))
            m1pool = ctx.enter_context(tc.tile_pool(name="mem1", bufs=2))
            m2pool = ctx.enter_context(tc.tile_pool(name="mem2", bufs=2))
            s1pool = ctx.enter_context(tc.tile_pool(name="spk1", bufs=2))
            s2pool = ctx.enter_context(tc.tile_pool(name="spk2", bufs=2))
            pspool = ctx.enter_context(
                tc.tile_pool(name="psum", bufs=6, space="PSUM"))

            # --- weights / biases (resident) ---
            w1h = wpool.tile([128, KT1, H], F16, tag="w1h")
            w1l = wpool.tile([128, KT1, H], F16, tag="w1l")
            w2h = wpool.tile([128, KT2, O], F16, tag="w2h")
            w2l = wpool.tile([128, KT2, O], F16, tag="w2l")
            b1s = wpool.tile([128, MT1], F32, tag="b1s")
            b2s = wpool.tile([128, MT2], F32, tag="b2s")
            # Interleave weight K-slices with block-0 x K-slices so the first
            # matmul group is gated by ~1 slice of DMA, not 4.5 MB of weights.
            nb0 = BLOCKS[0][1] * B_LOC
            xh0 = xpool.tile([128, KT1, nb0], F16, tag="xh")
            xl0 = xpool.tile([128, KT1, nb0], F16, tag="xl")
            for k in range(KT1):
                nc.sync.dma_start(
                    out=w1h[:, k, :],
                    in_=w1h_d.ap().rearrange("(k p) n -> p k n", p=128)[:, k, :])
                nc.sync.dma_start(
                    out=xh0[:, k, :],
                    in_=xh_d.ap().rearrange("(k p) n -> p k n", p=128)
                    [:, k, 0:nb0])
                nc.sync.dma_start(
                    out=xl0[:, k, :],
                    in_=xl_d.ap().rearrange("(k p) n -> p k n", p=128)
                    [:, k, 0:nb0])
                if k == 0:
                    nc.sync.dma_start(out=b1s[:, :],
                                      in_=b1_d.ap().rearrange("m p -> p m"))
                    nc.sync.dma_start(out=b2s[:, :],
                                      in_=b2_d.ap().rearrange("m p -> p m"))
            # w1l only becomes critical two passes into the first m-group
            for k in range(KT1):
                nc.sync.dma_start(
                    out=w1l[:, k, :],
                    in_=w1l_d.ap().rearrange("(k p) n -> p k n", p=128)[:, k, :])

            # --- initial membrane state (mem = 0) ---
            m1_init = wpool.tile([128, MT1, B_LOC], F32, tag="m1i")
            m2_init = wpool.tile([128, MT2, B_LOC], F32, tag="m2i")
            nc.vector.memset(m1_init[:, :, :], 0.0)
            nc.vector.memset(m2_init[:, :, :], 0.0)

            blk = {}   # j -> (xh, xl, cur1)
            prev = {}  # j -> (M1, M2, ncols)

            def emit_front(j):
                """DMA x block j; matmul-1 (+bias) -> cur1(j)."""
                c0, ns = BLOCKS[j]
                nb = ns * B_LOC
                if j == 0:
                    xh, xl = xh0, xl0
                else:
                    xh = xpool.tile([128, KT1, nb], F16, tag="xh")
                    xl = xpool.tile([128, KT1, nb], F16, tag="xl")
                    for k in range(KT1):
                        nc.sync.dma_start(
                            out=xh[:, k, :],
                            in_=xh_d.ap().rearrange("(k p) n -> p k n", p=128)
                            [:, k, c0:c0 + nb])
                        nc.sync.dma_start(
                            out=xl[:, k, :],
                            in_=xl_d.ap().rearrange("(k p) n -> p k n", p=128)
                            [:, k, c0:c0 + nb])
                cur1 = c1pool.tile([128, MT1, nb], F32, tag="cur1")
                for m in range(MT1):
                    ps = pspool.tile([128, nb], F32, tag="ps")
                    n_mm = 3 * KT1
                    i_mm = 0
                    for (wt, xt) in ((w1h, xh), (w1h, xl), (w1l, xh)):
                        for k in range(KT1):
                            nc.tensor.matmul(
                                ps[:, :],
                                wt[:, k, m * 128:(m + 1) * 128],
                                xt[:, k, :],
                                start=(i_mm == 0), stop=(i_mm == n_mm - 1))
                            i_mm += 1
                    nc.scalar.activation(cur1[:, m, :], ps[:, :], AF.Identity,
                                         bias=b1s[:, m:m + 1], scale=1.0)
                blk[j] = (xh, xl, cur1)

            def emit_back(j):
                """Scans + spike extraction + matmul-2 + output DMA, block j.
                The last block runs in sub-pieces to shorten the serial
                drain chain at the end of the kernel."""
                c0, ns = BLOCKS[j]
                nb = ns * B_LOC
                cur1 = blk.pop(j)[2]
                M1 = m1pool.tile([128, MT1, nb], F32, tag="M1")
                S1 = s1pool.tile([128, MT1, nb], F16, tag="S1")
                cur2 = c2pool.tile([128, MT2, nb], F32, tag="cur2")
                M2 = m2pool.tile([128, MT2, nb], F32, tag="M2")
                S2 = s2pool.tile([128, MT2, nb], F16, tag="S2")
                if j < len(BLOCKS) - 1:
                    pieces = [(0, ns)]
                else:
                    pieces = [(0, ns // 2), (ns // 2, ns)]
                # layer-1 scans + spike extraction + matmul-2, per piece
                # (all layer-1 pieces first so the DVE stream never blocks
                # on a later piece's matmul-2)
                for (sa, sb) in pieces:
                    ca, cb = sa * B_LOC, sb * B_LOC
                    for st in range(sa, sb):
                        lo, hi = st * B_LOC, (st + 1) * B_LOC
                        if st == 0:
                            pM1, _, pnb = (None, None, None) if j == 0 \
                                else prev[j - 1]
                            mp = m1_init[:, :, :] if j == 0 \
                                else pM1[:, :, pnb - B_LOC:pnb]
                        else:
                            mp = M1[:, :, lo - B_LOC:lo]
                        nc.vector._custom_dve(
                            LIF, out=M1[:, :, lo:hi], in0=mp,
                            in1=cur1[:, :, lo:hi], s0=BETA, s1=thr1)
                    # spikes (bulk) -> fp16 {0,1}; feeds mm2 and output DMA
                    nc.vector.tensor_scalar(S1[:, :, ca:cb], M1[:, :, ca:cb],
                                            thr1, None, AluOpType.is_gt)
                    # matmul-2 (+bias) -> cur2
                    for m in range(MT2):
                        ps = pspool.tile([128, cb - ca], F32, tag="ps")
                        n_mm = 2 * KT2
                        i_mm = 0
                        for wt in (w2h, w2l):
                            for k in range(KT2):
                                nc.tensor.matmul(
                                    ps[:, :],
                                    wt[:, k, m * 128:(m + 1) * 128],
                                    S1[:, k, ca:cb],
                                    start=(i_mm == 0), stop=(i_mm == n_mm - 1))
                                i_mm += 1
                        nc.scalar.activation(cur2[:, m, ca:cb], ps[:, :],
                                             AF.Identity,
                                             bias=b2s[:, m:m + 1], scale=1.0)
                    nc.sync.dma_start(
                        out=s1_d.ap().rearrange("k p n -> p k n")
                        [:, :, c0 + ca:c0 + cb],
                        in_=S1[:, :, ca:cb])
                # layer-2 scans + spike extraction + output DMA, per piece
                for (sa, sb) in pieces:
                    ca, cb = sa * B_LOC, sb * B_LOC
                    for st in range(sa, sb):
                        lo, hi = st * B_LOC, (st + 1) * B_LOC
                        if st == 0:
                            pM2 = None if j == 0 else prev[j - 1][1]
                            pnb = None if j == 0 else prev[j - 1][2]
                            mp = m2_init[:, :, :] if j == 0 \
                                else pM2[:, :, pnb - B_LOC:pnb]
                        else:
                            mp = M2[:, :, lo - B_LOC:lo]
                        nc.vector._custom_dve(
                            LIF, out=M2[:, :, lo:hi], in0=mp,
                            in1=cur2[:, :, lo:hi], s0=BETA, s1=thr2)
                    nc.vector.tensor_scalar(S2[:, :, ca:cb], M2[:, :, ca:cb],
                                            thr2, None, AluOpType.is_gt)
                    nc.sync.dma_start(
                        out=s2_d.ap().rearrange("k p n -> p k n")
                        [:, :, c0 + ca:c0 + cb],
                        in_=S2[:, :, ca:cb])
                prev.pop(j - 1, None)
                prev[j] = (M1, M2, nb)

            for j in range(len(BLOCKS)):
                emit_front(j)
                if j == 1:
                    # w2 weights are first needed by matmul-2 of block 0,
                    # well after the front-critical DMAs above
                    nc.sync.dma_start(
                        out=w2h[:, :, :],
                        in_=w2h_d.ap().rearrange("(k p) n -> p k n", p=128))
                    nc.sync.dma_start(
                        out=w2l[:, :, :],
                        in_=w2l_d.ap().rearrange("(k p) n -> p k n", p=128))
                if j >= 1:
                    emit_back(j - 1)
            emit_back(len(BLOCKS) - 1)

    nc.compile()
    return nc


_CACHE = {}


def _get_nc(thr1: float, thr2: float):
    key = (thr1, thr2)
    if key not in _CACHE:
        _CACHE[key] = _build(thr1, thr2)
    return _CACHE[key]


def _split_f16(a: np.ndarray):
    hi = a.astype(np.float16)
    lo = (a - hi.astype(np.float32)).astype(np.float16)
    return hi, lo


def kernel(x, w1, b1, w2, b2, thr1, thr2):
    global LAST_EXEC_NS
    x = np.asarray(x, np.float32)
    w1 = np.asarray(w1, np.float32)
    b1 = np.asarray(b1, np.float32)
    w2 = np.asarray(w2, np.float32)
    b2 = np.asarray(b2, np.float32)
    t1 = float(np.asarray(thr1))
    t2 = float(np.asarray(thr2))

    nc = _get_nc(t1, t2)

    # host-side prep: transpose + fp16 hi/lo split (weights replicated)
    w1h, w1l = _split_f16(np.ascontiguousarray(w1.T))        # [I, H]
    w2h, w2l = _split_f16(np.ascontiguousarray(w2.T))        # [H, O]
    b1r = np.ascontiguousarray(b1.reshape(MT1, 128))
    b2r = np.ascontiguousarray(b2.reshape(MT2, 128))

    in_maps = []
    for c in range(NCORES):
        xc = x[:, c * B_LOC:(c + 1) * B_LOC, :].reshape(TB, I)
        xh, xl = _split_f16(np.ascontiguousarray(xc.T))      # [I, TB]
        in_maps.append({
            "xh": xh, "xl": xl,
            "w1h": w1h, "w1l": w1l, "w2h": w2h, "w2l": w2l,
            "b1": b1r, "b2": b2r,
        })

    trace = bool(int(os.environ.get("KERNEL_TRACE", "0")))
    res = run_bass_kernel_spmd(nc, in_maps, core_ids=list(range(NCORES)),
                               trace=trace)
    LAST_EXEC_NS = res.exec_time_ns

    spk1 = np.empty((T, B, H), np.float32)
    spk2 = np.empty((T, B, O), np.float32)
    for c in range(NCORES):
        o1 = res.results[c]["s1"].astype(np.float32).reshape(H, TB)
        spk1[:, c * B_LOC:(c + 1) * B_LOC, :] = o1.T.reshape(T, B_LOC, H)
        o2 = res.results[c]["s2"].astype(np.float32).reshape(O, TB)
        spk2[:, c * B_LOC:(c + 1) * B_LOC, :] = o2.T.reshape(T, B_LOC, O)
    return spk1, spk2


# revision 27
# speedup vs baseline: 1.0160x; 1.0160x over previous
"""Trainium2 Bass kernel for the two-layer LIF spiking network.

Model (snntorch Leaky, reset_mechanism='subtract', beta=0.9):
    mem1 = beta*mem1 + (x_t @ w1.T + b1) - spk1_prev*thr1 ; spk1 = (mem1-thr1 > 0)
    mem2 = beta*mem2 + (spk1 @ w2.T + b2) - spk2_prev*thr2 ; spk2 = (mem2-thr2 > 0)

Strategy (8 NeuronCores, data-parallel over batch, B_loc = 16 per core):
  * Layer-1 currents for ALL timesteps are one big matmul (x known ahead);
    layer-2 currents for a time-block are a matmul of that block's spikes.
    Only the cheap elementwise membrane update is sequential.
  * f32 matmul on PE runs at 1/4 rate; instead split operands to fp16 hi+lo
    on the host and run 3 fp16 passes (wh*xh + wl*xh + wh*xl): ~2e-6 error
    at 3/4 of the f32 cost. Spikes are exactly representable in fp16, so
    matmul-2 needs only 2 passes (w2h, w2l).
  * The membrane scan is ONE custom fused DVE op per timestep:
        m' = (m*beta + cur) - (m > thr)*thr
    which is bit-exact the reference's f32 rounding order. Spikes are
    extracted per-block in bulk (is_gt) straight into the fp16 tile that
    feeds matmul-2 and the output DMA.
  * Layout: [128(par), slot, t*B_loc] slabs everywhere; matmul with K=i (or
    h) on partitions produces [h, (t,b)] directly; scans walk 16-column
    (one timestep) slices. No transposes anywhere; host reassembles.
  * Emission is software-pipelined (mm1 of block j+1 before mm2 of block j)
    so the PE stream never stalls on the scans.
"""

import os
import sys
import types

import numpy as np

_TRN_REPO = "/opt/trn_rl_repo"
if _TRN_REPO not in sys.path:
    sys.path.insert(0, _TRN_REPO)


def _install_axon_ntff_hook():
    """The image's antenv package lacks axon_hooks; provide it so
    run_bass_kernel_spmd(trace=True) can capture NTFF profiles."""
    if "antenv.axon_hooks" in sys.modules:
        return
    try:
        import antenv
    except ImportError:
        return
    mod = types.ModuleType("antenv.axon_hooks")
    mod._HOOK = None

    def set_axon_ntff_profile_hook(hook):
        mod._HOOK = hook

    def get_axon_ntff_profile_hook():
        if mod._HOOK is None:
            try:
                from trn_agent_boot.trn_boot import _ntff_profile_via_ctypes

                mod._HOOK = _ntff_profile_via_ctypes("/opt/axon/libaxon_pjrt.so")
            except Exception:
                mod._HOOK = None
        return mod._HOOK

    mod.set_axon_ntff_profile_hook = set_axon_ntff_profile_hook
    mod.get_axon_ntff_profile_hook = get_axon_ntff_profile_hook
    sys.modules["antenv.axon_hooks"] = mod
    antenv.axon_hooks = mod


_install_axon_ntff_hook()

from concourse import bacc, mybir, tile  # noqa: E402
from concourse.alu_op_type import AluOpType  # noqa: E402
from concourse.bass_utils import run_bass_kernel_spmd  # noqa: E402

AF = mybir.ActivationFunctionType
F16 = mybir.dt.float16
F32 = mybir.dt.float32

# Problem constants (hardcoded per spec).
T, B, I, H, O = 256, 128, 1024, 1024, 256
BETA = 0.9
NCORES = 8
B_LOC = B // NCORES            # 16
TB = T * B_LOC                 # 4096 columns (t-major, b-minor)
KT1, MT1 = I // 128, H // 128  # 8, 8
KT2, MT2 = H // 128, O // 128  # 8, 2
BLK_STEPS = 32                 # timesteps per standard pipeline block
BLK = BLK_STEPS * B_LOC        # 512 columns per block
NBLK = T // BLK_STEPS          # 8
# The first two blocks are half-size so the first matmul group is gated by
# ~3 MB of DMA instead of ~6 MB; the rest are full 32-step blocks.
_SIZES = [16, 16] + [32] * 6 + [16, 8, 8]
BLOCKS = [(sum(_SIZES[:i]) * B_LOC, _SIZES[i]) for i in range(len(_SIZES))]
assert sum(_SIZES) == T

LAST_EXEC_NS = None


def _register_lif_op():
    """Custom DVE op: out = (in0*s0 + in1) - (in0 > s1)*s1.

    With s0=beta, s1=thr and in1 the (bias-included) input current this is
    one LIF membrane update, with the same fp32 rounding order as the
    reference (verified bit-exact on HW)."""
    import concourse.dve_ops as dve_ops_mod
    from concourse.dve_spec import C0, C1, Spec, Src0, Src1, lower
    from concourse.dve_uop import DveOpSpec

    name = "LIF_STEP_ANT"
    for op in dve_ops_mod.OPS:
        if op.name == name:
            return op
    body = (Src0 * C0 + Src1) - (Src0 > C1) * C1

    def ref(in0, in1, s0, s1, imm2):
        return (in0.astype(np.float32) * s0 + in1) - (
            in0 > s1).astype(np.float32) * s1

    spec = Spec(body=body, reference=ref)
    shas = {}
    for ver in ("v3", "v4"):
        tmp = DveOpSpec(name=name, opcode=None, uops=lower(spec, ver=ver),
                        rd1_en=True)
        shas[ver] = tmp.sha(ver)
    op = dve_ops_mod.DveOp(name, spec, subdim=False, uops_sha=shas)
    row = max(dve_ops_mod._SUB_OPCODE_FOR_NAME.values()) + 1
    assert row < 0x20
    dve_ops_mod.OPS.append(op)
    dve_ops_mod._SUB_OPCODE_FOR_NAME[name] = row
    dve_ops_mod.CUSTOM_DVE_SPECS[name] = spec
    return op


LIF = _register_lif_op()


def _build(thr1: float, thr2: float):
    nc = bacc.Bacc("TRN2", target_bir_lowering=False, debug=False,
                   num_devices=NCORES)

    xh_d = nc.dram_tensor("xh", [I, TB], F16, kind="ExternalInput")
    xl_d = nc.dram_tensor("xl", [I, TB], F16, kind="ExternalInput")
    w1h_d = nc.dram_tensor("w1h", [I, H], F16, kind="ExternalInput")
    w1l_d = nc.dram_tensor("w1l", [I, H], F16, kind="ExternalInput")
    w2h_d = nc.dram_tensor("w2h", [H, O], F16, kind="ExternalInput")
    w2l_d = nc.dram_tensor("w2l", [H, O], F16, kind="ExternalInput")
    b1_d = nc.dram_tensor("b1", [MT1, 128], F32, kind="ExternalInput")
    b2_d = nc.dram_tensor("b2", [MT2, 128], F32, kind="ExternalInput")
    s1_d = nc.dram_tensor("s1", [MT1, 128, TB], F16, kind="ExternalOutput")
    s2_d = nc.dram_tensor("s2", [MT2, 128, TB], F16, kind="ExternalOutput")

    with tile.TileContext(nc) as tc:
        import contextlib

        ctx = contextlib.ExitStack()
        with ctx:
            wpool = ctx.enter_context(tc.tile_pool(name="weights", bufs=1))
            xpool = ctx.enter_context(tc.tile_pool(name="x", bufs=3))
            c1pool = ctx.enter_context(tc.tile_pool(name="cur1", bufs=2))
            c2pool = ctx.enter_context(tc.tile_pool(name="cur2", bufs=2))
            m1pool = ctx.enter_context(tc.tile_pool(name="mem1", bufs=2))
            m2pool = ctx.enter_context(tc.tile_pool(name="mem2", bufs=2))
            s1pool = ctx.enter_context(tc.tile_pool(name="spk1", bufs=2))
            s2pool = ctx.enter_context(tc.tile_pool(name="spk2", bufs=2))
            pspool = ctx.enter_context(
                tc.tile_pool(name="psum", bufs=4, space="PSUM"))

            # --- weights / biases (resident) ---
            w1h = wpool.tile([128, KT1, H], F16, tag="w1h")
            w1l = wpool.tile([128, KT1, H], F16, tag="w1l")
            w2h = wpool.tile([128, KT2, O], F16, tag="w2h")
            w2l = wpool.tile([128, KT2, O], F16, tag="w2l")
            b1s = wpool.tile([128, MT1], F32, tag="b1s")
            b2s = wpool.tile([128, MT2], F32, tag="b2s")
            # Interleave weight K-slices with block-0 x K-slices so the first
            # matmul group is gated by ~1 slice of DMA, not 4.5 MB of weights.
            nb0 = BLOCKS[0][1] * B_LOC
            xh0 = xpool.tile([128, KT1, nb0], F16, tag="xh")
            xl0 = xpool.tile([128, KT1, nb0], F16, tag="xl")
            for k in range(KT1):
                nc.sync.dma_start(
                    out=w1h[:, k, :],
                    in_=w1h_d.ap().rearrange("(k p) n -> p k n", p=128)[:, k, :])
                nc.sync.dma_start(
                    out=xh0[:, k, :],
                    in_=xh_d.ap().rearrange("(k p) n -> p k n", p=128)
                    [:, k, 0:nb0])
                nc.sync.dma_start(
                    out=xl0[:, k, :],
                    in_=xl_d.ap().rearrange("(k p) n -> p k n", p=128)
                    [:, k, 0:nb0])
                if k == 0:
                    nc.sync.dma_start(out=b1s[:, :],
                                      in_=b1_d.ap().rearrange("m p -> p m"))
                    nc.sync.dma_start(out=b2s[:, :],
                                      in_=b2_d.ap().rearrange("m p -> p m"))
            # w1l only becomes critical two passes into the first m-group
            for k in range(KT1):
                nc.sync.dma_start(
                    out=w1l[:, k, :],
                    in_=w1l_d.ap().rearrange("(k p) n -> p k n", p=128)[:, k, :])

            # --- initial membrane state (mem = 0) ---
            m1_init = wpool.tile([128, MT1, B_LOC], F32, tag="m1i")
            m2_init = wpool.tile([128, MT2, B_LOC], F32, tag="m2i")
            nc.vector.memset(m1_init[:, :, :], 0.0)
            nc.vector.memset(m2_init[:, :, :], 0.0)

            blk = {}   # j -> (xh, xl, cur1)
            prev = {}  # j -> (M1, M2, ncols)

            def emit_front(j):
                """DMA x block j; matmul-1 (+bias) -> cur1(j)."""
                c0, ns = BLOCKS[j]
                nb = ns * B_LOC
                if j == 0:
                    xh, xl = xh0, xl0
                else:
                    xh = xpool.tile([128, KT1, nb], F16, tag="xh")
                    xl = xpool.tile([128, KT1, nb], F16, tag="xl")
                    for k in range(KT1):
                        nc.sync.dma_start(
                            out=xh[:, k, :],
                            in_=xh_d.ap().rearrange("(k p) n -> p k n", p=128)
                            [:, k, c0:c0 + nb])
                        nc.sync.dma_start(
                            out=xl[:, k, :],
                            in_=xl_d.ap().rearrange("(k p) n -> p k n", p=128)
                            [:, k, c0:c0 + nb])
                cur1 = c1pool.tile([128, MT1, nb], F32, tag="cur1")
                for m in range(MT1):
                    ps = pspool.tile([128, nb], F32, tag="ps")
                    n_mm = 3 * KT1
                    i_mm = 0
                    for (wt, xt) in ((w1h, xh), (w1h, xl), (w1l, xh)):
                        for k in range(KT1):
                            nc.tensor.matmul(
                                ps[:, :],
                                wt[:, k, m * 128:(m + 1) * 128],
                                xt[:, k, :],
                                start=(i_mm == 0), stop=(i_mm == n_mm - 1))
                            i_mm += 1
                    nc.scalar.activation(cur1[:, m, :], ps[:, :], AF.Identity,
                                         bias=b1s[:, m:m + 1], scale=1.0)
                blk[j] = (xh, xl, cur1)

            def emit_back(j):
                """Scans + spike extraction + matmul-2 + output DMA, block j.
                The last block runs in sub-pieces to shorten the serial
                drain chain at the end of the kernel."""
                c0, ns = BLOCKS[j]
                nb = ns * B_LOC
                cur1 = blk.pop(j)[2]
                M1 = m1pool.tile([128, MT1, nb], F32, tag="M1")
                S1 = s1pool.tile([128, MT1, nb], F16, tag="S1")
                cur2 = c2pool.tile([128, MT2, nb], F32, tag="cur2")
                M2 = m2pool.tile([128, MT2, nb], F32, tag="M2")
                S2 = s2pool.tile([128, MT2, nb], F16, tag="S2")
                if j < len(BLOCKS) - 1:
                    pieces = [(0, ns)]
                else:
                    pieces = [(0, ns // 2), (ns // 2, ns)]
                # layer-1 scans + spike extraction + matmul-2, per piece
                # (all layer-1 pieces first so the DVE stream never blocks
                # on a later piece's matmul-2)
                for (sa, sb) in pieces:
                    ca, cb = sa * B_LOC, sb * B_LOC
                    for st in range(sa, sb):
                        lo, hi = st * B_LOC, (st + 1) * B_LOC
                        if st == 0:
                            pM1, _, pnb = (None, None, None) if j == 0 \
                                else prev[j - 1]
                            mp = m1_init[:, :, :] if j == 0 \
                                else pM1[:, :, pnb - B_LOC:pnb]
                        else:
                            mp = M1[:, :, lo - B_LOC:lo]
                        nc.vector._custom_dve(
                            LIF, out=M1[:, :, lo:hi], in0=mp,
                            in1=cur1[:, :, lo:hi], s0=BETA, s1=thr1)
                    # spikes (bulk) -> fp16 {0,1}; feeds mm2 and output DMA
                    nc.vector.tensor_scalar(S1[:, :, ca:cb], M1[:, :, ca:cb],
                                            thr1, None, AluOpType.is_gt)
                    # matmul-2 (+bias) -> cur2
                    for m in range(MT2):
                        ps = pspool.tile([128, cb - ca], F32, tag="ps")
                        n_mm = 2 * KT2
                        i_mm = 0
                        for wt in (w2h, w2l):
                            for k in range(KT2):
                                nc.tensor.matmul(
                                    ps[:, :],
                                    wt[:, k, m * 128:(m + 1) * 128],
                                    S1[:, k, ca:cb],
                                    start=(i_mm == 0), stop=(i_mm == n_mm - 1))
                                i_mm += 1
                        nc.scalar.activation(cur2[:, m, ca:cb], ps[:, :],
                                             AF.Identity,
                                             bias=b2s[:, m:m + 1], scale=1.0)
                    nc.sync.dma_start(
                        out=s1_d.ap().rearrange("k p n -> p k n")
                        [:, :, c0 + ca:c0 + cb],
                        in_=S1[:, :, ca:cb])
                # layer-2 scans + spike extraction + output DMA, per piece
                for (sa, sb) in pieces:
                    ca, cb = sa * B_LOC, sb * B_LOC
                    for st in range(sa, sb):
                        lo, hi = st * B_LOC, (st + 1) * B_LOC
                        if st == 0:
                            pM2 = None if j == 0 else prev[j - 1][1]
                            pnb = None if j == 0 else prev[j - 1][2]
                            mp = m2_init[:, :, :] if j == 0 \
                                else pM2[:, :, pnb - B_LOC:pnb]
                        else:
                            mp = M2[:, :, lo - B_LOC:lo]
                        nc.vector._custom_dve(
                            LIF, out=M2[:, :, lo:hi], in0=mp,
                            in1=cur2[:, :, lo:hi], s0=BETA, s1=thr2)
                    nc.vector.tensor_scalar(S2[:, :, ca:cb], M2[:, :, ca:cb],
                                            thr2, None, AluOpType.is_gt)
                    nc.sync.dma_start(
                        out=s2_d.ap().rearrange("k p n -> p k n")
                        [:, :, c0 + ca:c0 + cb],
                        in_=S2[:, :, ca:cb])
                prev.pop(j - 1, None)
                prev[j] = (M1, M2, nb)

            for j in range(len(BLOCKS)):
                emit_front(j)
                if j == 1:
                    # w2 weights are first needed by matmul-2 of block 0,
                    # well after the front-critical DMAs above
                    nc.sync.dma_start(
                        out=w2h[:, :, :],
                        in_=w2h_d.ap().rearrange("(k p) n -> p k n", p=128))
                    nc.sync.dma_start(
                        out=w2l[:, :, :],
                        in_=w2l_d.ap().rearrange("(k p) n -> p k n", p=128))
                if j >= 1:
                    emit_back(j - 1)
            emit_back(len(BLOCKS) - 1)

    nc.compile()
    return nc


_CACHE = {}


def _get_nc(thr1: float, thr2: float):
    key = (thr1, thr2)
    if key not in _CACHE:
        _CACHE[key] = _build(thr1, thr2)
    return _CACHE[key]


def _split_f16(a: np.ndarray):
    hi = a.astype(np.float16)
    lo = (a - hi.astype(np.float32)).astype(np.float16)
    return hi, lo


def kernel(x, w1, b1, w2, b2, thr1, thr2):
    global LAST_EXEC_NS
    x = np.asarray(x, np.float32)
    w1 = np.asarray(w1, np.float32)
    b1 = np.asarray(b1, np.float32)
    w2 = np.asarray(w2, np.float32)
    b2 = np.asarray(b2, np.float32)
    t1 = float(np.asarray(thr1))
    t2 = float(np.asarray(thr2))

    nc = _get_nc(t1, t2)

    # host-side prep: transpose + fp16 hi/lo split (weights replicated)
    w1h, w1l = _split_f16(np.ascontiguousarray(w1.T))        # [I, H]
    w2h, w2l = _split_f16(np.ascontiguousarray(w2.T))        # [H, O]
    b1r = np.ascontiguousarray(b1.reshape(MT1, 128))
    b2r = np.ascontiguousarray(b2.reshape(MT2, 128))

    in_maps = []
    for c in range(NCORES):
        xc = x[:, c * B_LOC:(c + 1) * B_LOC, :].reshape(TB, I)
        xh, xl = _split_f16(np.ascontiguousarray(xc.T))      # [I, TB]
        in_maps.append({
            "xh": xh, "xl": xl,
            "w1h": w1h, "w1l": w1l, "w2h": w2h, "w2l": w2l,
            "b1": b1r, "b2": b2r,
        })

    trace = bool(int(os.environ.get("KERNEL_TRACE", "0")))
    res = run_bass_kernel_spmd(nc, in_maps, core_ids=list(range(NCORES)),
                               trace=trace)
    LAST_EXEC_NS = res.exec_time_ns

    spk1 = np.empty((T, B, H), np.float32)
    spk2 = np.empty((T, B, O), np.float32)
    for c in range(NCORES):
        o1 = res.results[c]["s1"].astype(np.float32).reshape(H, TB)
        spk1[:, c * B_LOC:(c + 1) * B_LOC, :] = o1.T.reshape(T, B_LOC, H)
        o2 = res.results[c]["s2"].astype(np.float32).reshape(O, TB)
        spk2[:, c * B_LOC:(c + 1) * B_LOC, :] = o2.T.reshape(T, B_LOC, O)
    return spk1, spk2


# revision 28
# speedup vs baseline: 1.0195x; 1.0035x over previous
"""Trainium2 Bass kernel for the two-layer LIF spiking network.

Model (snntorch Leaky, reset_mechanism='subtract', beta=0.9):
    mem1 = beta*mem1 + (x_t @ w1.T + b1) - spk1_prev*thr1 ; spk1 = (mem1-thr1 > 0)
    mem2 = beta*mem2 + (spk1 @ w2.T + b2) - spk2_prev*thr2 ; spk2 = (mem2-thr2 > 0)

Strategy (8 NeuronCores, data-parallel over batch, B_loc = 16 per core):
  * Layer-1 currents for ALL timesteps are one big matmul (x known ahead);
    layer-2 currents for a time-block are a matmul of that block's spikes.
    Only the cheap elementwise membrane update is sequential.
  * f32 matmul on PE runs at 1/4 rate; instead split operands to fp16 hi+lo
    on the host and run 3 fp16 passes (wh*xh + wl*xh + wh*xl): ~2e-6 error
    at 3/4 of the f32 cost. Spikes are exactly representable in fp16, so
    matmul-2 needs only 2 passes (w2h, w2l).
  * The membrane scan is ONE custom fused DVE op per timestep:
        m' = (m*beta + cur) - (m > thr)*thr
    which is bit-exact the reference's f32 rounding order. Spikes are
    extracted per-block in bulk (is_gt) straight into the fp16 tile that
    feeds matmul-2 and the output DMA.
  * Layout: [128(par), slot, t*B_loc] slabs everywhere; matmul with K=i (or
    h) on partitions produces [h, (t,b)] directly; scans walk 16-column
    (one timestep) slices. No transposes anywhere; host reassembles.
  * Emission is software-pipelined (mm1 of block j+1 before mm2 of block j)
    so the PE stream never stalls on the scans.
"""

import os
import sys
import types

import numpy as np

_TRN_REPO = "/opt/trn_rl_repo"
if _TRN_REPO not in sys.path:
    sys.path.insert(0, _TRN_REPO)


def _install_axon_ntff_hook():
    """The image's antenv package lacks axon_hooks; provide it so
    run_bass_kernel_spmd(trace=True) can capture NTFF profiles."""
    if "antenv.axon_hooks" in sys.modules:
        return
    try:
        import antenv
    except ImportError:
        return
    mod = types.ModuleType("antenv.axon_hooks")
    mod._HOOK = None

    def set_axon_ntff_profile_hook(hook):
        mod._HOOK = hook

    def get_axon_ntff_profile_hook():
        if mod._HOOK is None:
            try:
                from trn_agent_boot.trn_boot import _ntff_profile_via_ctypes

                mod._HOOK = _ntff_profile_via_ctypes("/opt/axon/libaxon_pjrt.so")
            except Exception:
                mod._HOOK = None
        return mod._HOOK

    mod.set_axon_ntff_profile_hook = set_axon_ntff_profile_hook
    mod.get_axon_ntff_profile_hook = get_axon_ntff_profile_hook
    sys.modules["antenv.axon_hooks"] = mod
    antenv.axon_hooks = mod


_install_axon_ntff_hook()

from concourse import bacc, mybir, tile  # noqa: E402
from concourse.alu_op_type import AluOpType  # noqa: E402
from concourse.bass_utils import run_bass_kernel_spmd  # noqa: E402

AF = mybir.ActivationFunctionType
F16 = mybir.dt.float16
F32 = mybir.dt.float32

# Problem constants (hardcoded per spec).
T, B, I, H, O = 256, 128, 1024, 1024, 256
BETA = 0.9
NCORES = 8
B_LOC = B // NCORES            # 16
TB = T * B_LOC                 # 4096 columns (t-major, b-minor)
KT1, MT1 = I // 128, H // 128  # 8, 8
KT2, MT2 = H // 128, O // 128  # 8, 2
BLK_STEPS = 32                 # timesteps per standard pipeline block
BLK = BLK_STEPS * B_LOC        # 512 columns per block
NBLK = T // BLK_STEPS          # 8
# The first two blocks are half-size so the first matmul group is gated by
# ~3 MB of DMA instead of ~6 MB; the rest are full 32-step blocks.
_SIZES = [16, 16] + [32] * 6 + [16, 8, 8]
BLOCKS = [(sum(_SIZES[:i]) * B_LOC, _SIZES[i]) for i in range(len(_SIZES))]
assert sum(_SIZES) == T

LAST_EXEC_NS = None


def _register_lif_op():
    """Custom DVE op: out = (in0*s0 + in1) - (in0 > s1)*s1.

    With s0=beta, s1=thr and in1 the (bias-included) input current this is
    one LIF membrane update, with the same fp32 rounding order as the
    reference (verified bit-exact on HW)."""
    import concourse.dve_ops as dve_ops_mod
    from concourse.dve_spec import C0, C1, Spec, Src0, Src1, lower
    from concourse.dve_uop import DveOpSpec

    name = "LIF_STEP_ANT"
    for op in dve_ops_mod.OPS:
        if op.name == name:
            return op
    body = (Src0 * C0 + Src1) - (Src0 > C1) * C1

    def ref(in0, in1, s0, s1, imm2):
        return (in0.astype(np.float32) * s0 + in1) - (
            in0 > s1).astype(np.float32) * s1

    spec = Spec(body=body, reference=ref)
    shas = {}
    for ver in ("v3", "v4"):
        tmp = DveOpSpec(name=name, opcode=None, uops=lower(spec, ver=ver),
                        rd1_en=True)
        shas[ver] = tmp.sha(ver)
    op = dve_ops_mod.DveOp(name, spec, subdim=False, uops_sha=shas)
    row = max(dve_ops_mod._SUB_OPCODE_FOR_NAME.values()) + 1
    assert row < 0x20
    dve_ops_mod.OPS.append(op)
    dve_ops_mod._SUB_OPCODE_FOR_NAME[name] = row
    dve_ops_mod.CUSTOM_DVE_SPECS[name] = spec
    return op


LIF = _register_lif_op()


def _build(thr1: float, thr2: float):
    nc = bacc.Bacc("TRN2", target_bir_lowering=False, debug=False,
                   num_devices=NCORES)

    xh_d = nc.dram_tensor("xh", [I, TB], F16, kind="ExternalInput")
    xl_d = nc.dram_tensor("xl", [I, TB], F16, kind="ExternalInput")
    w1h_d = nc.dram_tensor("w1h", [I, H], F16, kind="ExternalInput")
    w1l_d = nc.dram_tensor("w1l", [I, H], F16, kind="ExternalInput")
    w2h_d = nc.dram_tensor("w2h", [H, O], F16, kind="ExternalInput")
    w2l_d = nc.dram_tensor("w2l", [H, O], F16, kind="ExternalInput")
    b1_d = nc.dram_tensor("b1", [MT1, 128], F32, kind="ExternalInput")
    b2_d = nc.dram_tensor("b2", [MT2, 128], F32, kind="ExternalInput")
    s1_d = nc.dram_tensor("s1", [MT1, 128, TB], F16, kind="ExternalOutput")
    s2_d = nc.dram_tensor("s2", [MT2, 128, TB], F16, kind="ExternalOutput")

    with tile.TileContext(nc) as tc:
        import contextlib

        ctx = contextlib.ExitStack()
        with ctx:
            wpool = ctx.enter_context(tc.tile_pool(name="weights", bufs=1))
            xpool = ctx.enter_context(tc.tile_pool(name="x", bufs=3))
            c1pool = ctx.enter_context(tc.tile_pool(name="cur1", bufs=2))
            c2pool = ctx.enter_context(tc.tile_pool(name="cur2", bufs=2))
            m1pool = ctx.enter_context(tc.tile_pool(name="mem1", bufs=2))
            m2pool = ctx.enter_context(tc.tile_pool(name="mem2", bufs=2))
            s1pool = ctx.enter_context(tc.tile_pool(name="spk1", bufs=2))
            s2pool = ctx.enter_context(tc.tile_pool(name="spk2", bufs=2))
            pspool = ctx.enter_context(
                tc.tile_pool(name="psum", bufs=6, space="PSUM"))

            # --- weights / biases (resident) ---
            w1h = wpool.tile([128, KT1, H], F16, tag="w1h")
            w1l = wpool.tile([128, KT1, H], F16, tag="w1l")
            w2h = wpool.tile([128, KT2, O], F16, tag="w2h")
            w2l = wpool.tile([128, KT2, O], F16, tag="w2l")
            b1s = wpool.tile([128, MT1], F32, tag="b1s")
            b2s = wpool.tile([128, MT2], F32, tag="b2s")
            # Interleave weight K-slices with block-0 x K-slices so the first
            # matmul group is gated by ~1 slice of DMA, not 4.5 MB of weights.
            nb0 = BLOCKS[0][1] * B_LOC
            xh0 = xpool.tile([128, KT1, nb0], F16, tag="xh")
            xl0 = xpool.tile([128, KT1, nb0], F16, tag="xl")
            for k in range(KT1):
                nc.sync.dma_start(
                    out=w1h[:, k, :],
                    in_=w1h_d.ap().rearrange("(k p) n -> p k n", p=128)[:, k, :])
                nc.sync.dma_start(
                    out=xh0[:, k, :],
                    in_=xh_d.ap().rearrange("(k p) n -> p k n", p=128)
                    [:, k, 0:nb0])
                nc.sync.dma_start(
                    out=xl0[:, k, :],
                    in_=xl_d.ap().rearrange("(k p) n -> p k n", p=128)
                    [:, k, 0:nb0])
                if k == 0:
                    nc.sync.dma_start(out=b1s[:, :],
                                      in_=b1_d.ap().rearrange("m p -> p m"))
                    nc.sync.dma_start(out=b2s[:, :],
                                      in_=b2_d.ap().rearrange("m p -> p m"))
            # w1l only becomes critical two passes into the first m-group
            for k in range(KT1):
                nc.sync.dma_start(
                    out=w1l[:, k, :],
                    in_=w1l_d.ap().rearrange("(k p) n -> p k n", p=128)[:, k, :])

            # --- initial membrane state (mem = 0) ---
            m1_init = wpool.tile([128, MT1, B_LOC], F32, tag="m1i")
            m2_init = wpool.tile([128, MT2, B_LOC], F32, tag="m2i")
            nc.vector.memset(m1_init[:, :, :], 0.0)
            nc.vector.memset(m2_init[:, :, :], 0.0)

            blk = {}   # j -> (xh, xl, cur1)
            prev = {}  # j -> (M1, M2, ncols)

            def emit_front(j):
                """DMA x block j; matmul-1 (+bias) -> cur1(j)."""
                c0, ns = BLOCKS[j]
                nb = ns * B_LOC
                if j == 0:
                    xh, xl = xh0, xl0
                else:
                    xh = xpool.tile([128, KT1, nb], F16, tag="xh")
                    xl = xpool.tile([128, KT1, nb], F16, tag="xl")
                    for k in range(KT1):
                        nc.sync.dma_start(
                            out=xh[:, k, :],
                            in_=xh_d.ap().rearrange("(k p) n -> p k n", p=128)
                            [:, k, c0:c0 + nb])
                        nc.sync.dma_start(
                            out=xl[:, k, :],
                            in_=xl_d.ap().rearrange("(k p) n -> p k n", p=128)
                            [:, k, c0:c0 + nb])
                cur1 = c1pool.tile([128, MT1, nb], F32, tag="cur1")
                for m in range(MT1):
                    ps = pspool.tile([128, nb], F32, tag="ps")
                    n_mm = 3 * KT1
                    i_mm = 0
                    for (wt, xt) in ((w1h, xh), (w1h, xl), (w1l, xh)):
                        for k in range(KT1):
                            nc.tensor.matmul(
                                ps[:, :],
                                wt[:, k, m * 128:(m + 1) * 128],
                                xt[:, k, :],
                                start=(i_mm == 0), stop=(i_mm == n_mm - 1))
                            i_mm += 1
                    nc.scalar.activation(cur1[:, m, :], ps[:, :], AF.Identity,
                                         bias=b1s[:, m:m + 1], scale=1.0)
                blk[j] = (xh, xl, cur1)

            def emit_back(j):
                """Scans + spike extraction + matmul-2 + output DMA, block j.
                The last block runs in sub-pieces to shorten the serial
                drain chain at the end of the kernel."""
                c0, ns = BLOCKS[j]
                nb = ns * B_LOC
                cur1 = blk.pop(j)[2]
                M1 = m1pool.tile([128, MT1, nb], F32, tag="M1")
                S1 = s1pool.tile([128, MT1, nb], F16, tag="S1")
                cur2 = c2pool.tile([128, MT2, nb], F32, tag="cur2")
                M2 = m2pool.tile([128, MT2, nb], F32, tag="M2")
                S2 = s2pool.tile([128, MT2, nb], F16, tag="S2")
                if j < len(BLOCKS) - 1:
                    pieces = [(0, ns)]
                else:
                    pieces = [(0, ns // 2), (ns // 2, ns)]
                # layer-1 scans + spike extraction + matmul-2, per piece
                # (all layer-1 pieces first so the DVE stream never blocks
                # on a later piece's matmul-2)
                for (sa, sb) in pieces:
                    ca, cb = sa * B_LOC, sb * B_LOC
                    for st in range(sa, sb):
                        lo, hi = st * B_LOC, (st + 1) * B_LOC
                        if st == 0:
                            pM1, _, pnb = (None, None, None) if j == 0 \
                                else prev[j - 1]
                            mp = m1_init[:, :, :] if j == 0 \
                                else pM1[:, :, pnb - B_LOC:pnb]
                        else:
                            mp = M1[:, :, lo - B_LOC:lo]
                        nc.vector._custom_dve(
                            LIF, out=M1[:, :, lo:hi], in0=mp,
                            in1=cur1[:, :, lo:hi], s0=BETA, s1=thr1)
                    # spikes (bulk) -> fp16 {0,1}; feeds mm2 and output DMA
                    nc.vector.tensor_scalar(S1[:, :, ca:cb], M1[:, :, ca:cb],
                                            thr1, None, AluOpType.is_gt)
                    # matmul-2 (+bias) -> cur2
                    for m in range(MT2):
                        ps = pspool.tile([128, cb - ca], F32, tag="ps")
                        n_mm = 2 * KT2
                        i_mm = 0
                        for wt in (w2h, w2l):
                            for k in range(KT2):
                                nc.tensor.matmul(
                                    ps[:, :],
                                    wt[:, k, m * 128:(m + 1) * 128],
                                    S1[:, k, ca:cb],
                                    start=(i_mm == 0), stop=(i_mm == n_mm - 1))
                                i_mm += 1
                        nc.scalar.activation(cur2[:, m, ca:cb], ps[:, :],
                                             AF.Identity,
                                             bias=b2s[:, m:m + 1], scale=1.0)
                    nc.sync.dma_start(
                        out=s1_d.ap().rearrange("k p n -> p k n")
                        [:, :, c0 + ca:c0 + cb],
                        in_=S1[:, :, ca:cb])
                # layer-2 scans + spike extraction + output DMA, per piece
                for (sa, sb) in pieces:
                    ca, cb = sa * B_LOC, sb * B_LOC
                    for st in range(sa, sb):
                        lo, hi = st * B_LOC, (st + 1) * B_LOC
                        if st == 0:
                            pM2 = None if j == 0 else prev[j - 1][1]
                            pnb = None if j == 0 else prev[j - 1][2]
                            mp = m2_init[:, :, :] if j == 0 \
                                else pM2[:, :, pnb - B_LOC:pnb]
                        else:
                            mp = M2[:, :, lo - B_LOC:lo]
                        nc.vector._custom_dve(
                            LIF, out=M2[:, :, lo:hi], in0=mp,
                            in1=cur2[:, :, lo:hi], s0=BETA, s1=thr2)
                    nc.vector.tensor_scalar(S2[:, :, ca:cb], M2[:, :, ca:cb],
                                            thr2, None, AluOpType.is_gt)
                    nc.sync.dma_start(
                        out=s2_d.ap().rearrange("k p n -> p k n")
                        [:, :, c0 + ca:c0 + cb],
                        in_=S2[:, :, ca:cb])
                prev.pop(j - 1, None)
                prev[j] = (M1, M2, nb)

            for j in range(len(BLOCKS)):
                emit_front(j)
                if j == 1:
                    # w2 weights are first needed by matmul-2 of block 0,
                    # well after the front-critical DMAs above
                    nc.sync.dma_start(
                        out=w2h[:, :, :],
                        in_=w2h_d.ap().rearrange("(k p) n -> p k n", p=128))
                    nc.sync.dma_start(
                        out=w2l[:, :, :],
                        in_=w2l_d.ap().rearrange("(k p) n -> p k n", p=128))
                if j >= 1:
                    emit_back(j - 1)
            emit_back(len(BLOCKS) - 1)

    nc.compile()
    return nc


_CACHE = {}


def _get_nc(thr1: float, thr2: float):
    key = (thr1, thr2)
    if key not in _CACHE:
        _CACHE[key] = _build(thr1, thr2)
    return _CACHE[key]


def _split_f16(a: np.ndarray):
    hi = a.astype(np.float16)
    lo = (a - hi.astype(np.float32)).astype(np.float16)
    return hi, lo


def kernel(x, w1, b1, w2, b2, thr1, thr2):
    global LAST_EXEC_NS
    x = np.asarray(x, np.float32)
    w1 = np.asarray(w1, np.float32)
    b1 = np.asarray(b1, np.float32)
    w2 = np.asarray(w2, np.float32)
    b2 = np.asarray(b2, np.float32)
    t1 = float(np.asarray(thr1))
    t2 = float(np.asarray(thr2))

    nc = _get_nc(t1, t2)

    # host-side prep: transpose + fp16 hi/lo split (weights replicated)
    w1h, w1l = _split_f16(np.ascontiguousarray(w1.T))        # [I, H]
    w2h, w2l = _split_f16(np.ascontiguousarray(w2.T))        # [H, O]
    b1r = np.ascontiguousarray(b1.reshape(MT1, 128))
    b2r = np.ascontiguousarray(b2.reshape(MT2, 128))

    in_maps = []
    for c in range(NCORES):
        xc = x[:, c * B_LOC:(c + 1) * B_LOC, :].reshape(TB, I)
        xh, xl = _split_f16(np.ascontiguousarray(xc.T))      # [I, TB]
        in_maps.append({
            "xh": xh, "xl": xl,
            "w1h": w1h, "w1l": w1l, "w2h": w2h, "w2l": w2l,
            "b1": b1r, "b2": b2r,
        })

    trace = bool(int(os.environ.get("KERNEL_TRACE", "0")))
    res = run_bass_kernel_spmd(nc, in_maps, core_ids=list(range(NCORES)),
                               trace=trace)
    LAST_EXEC_NS = res.exec_time_ns

    spk1 = np.empty((T, B, H), np.float32)
    spk2 = np.empty((T, B, O), np.float32)
    for c in range(NCORES):
        o1 = res.results[c]["s1"].astype(np.float32).reshape(H, TB)
        spk1[:, c * B_LOC:(c + 1) * B_LOC, :] = o1.T.reshape(T, B_LOC, H)
        o2 = res.results[c]["s2"].astype(np.float32).reshape(O, TB)
        spk2[:, c * B_LOC:(c + 1) * B_LOC, :] = o2.T.reshape(T, B_LOC, O)
    return spk1, spk2
